# revision 1
# baseline (speedup 1.0000x reference)
"""Fused Mllama-style text self-attention on one TRN2 chip (8 NeuronCores).

Sharding: tensor-parallel over heads (4 q heads / 1 kv head per core) for the
QKV projections + RoPE + attention; per-head AllToAlls reshard the attention
outputs to token-parallel, so each core computes a 512-token slice of the final
output projection against the full wo. Host code transposes operands so every
matmul contraction lands on the partition dimension, and re-assembles the
token-sliced outputs.

kernel(**inputs) takes the FULL (unsharded) inputs and returns the FULL output.
"""

import math

import numpy as np
import ml_dtypes

import concourse.bacc as bacc
import concourse.bass as bass
import concourse.mybir as mybir
import concourse.tile as tile
from concourse.bass_utils import run_bass_kernel_spmd

F32 = mybir.dt.float32
BF16 = mybir.dt.bfloat16
AF = mybir.ActivationFunctionType
ALU = mybir.AluOpType

NH, NKV, HD = 32, 8, 128
NEG = -1.0e9
N_CORES = 8


def build(T, S, H, compute_dtype="bf16", causal=True, n_cores=N_CORES):
    """Build the SPMD Bass program (same program for all cores).

    T: total tokens (B*S); S: seq len per batch; H: hidden size.
    """
    B = T // S
    TC = T // n_cores          # tokens per core in the output projection
    QHC = NH // n_cores        # local q heads (4)
    D = QHC * HD               # local q width (512)
    HT = H // 128              # contraction tiles over hidden
    QB = min(512, TC)          # attention query block width
    NQB = S // QB              # query blocks per batch
    KB = QB // 128             # 128-k-tiles per query block
    NKT = S // 128             # k tiles per batch
    NMB = H // 512             # output-projection column blocks
    NT = TC // 128             # output-projection row tiles
    CD = BF16 if compute_dtype == "bf16" else F32
    ISQ = 1.0 / math.sqrt(HD)
    TI = 512                   # tokens per QKV iteration
    NIT = T // TI

    nc = bacc.Bacc("TRN2", target_bir_lowering=False, debug=False,
                   enable_asserts=True, num_devices=n_cores)

    hsT = nc.declare_dram_parameter("hsT", [H, T], CD, isOutput=False)
    wqT = nc.declare_dram_parameter("wqT", [H, D], CD, isOutput=False)
    wkT = nc.declare_dram_parameter("wkT", [H, HD], CD, isOutput=False)
    wvT = nc.declare_dram_parameter("wvT", [H, HD], CD, isOutput=False)
    woT = nc.declare_dram_parameter("woT", [NH * HD, H], CD, isOutput=False)
    cosT = nc.declare_dram_parameter("cosT", [HD, S], F32, isOutput=False)
    sgnT = nc.declare_dram_parameter("sgnT", [HD, S], F32, isOutput=False)
    if causal:
        dmask = nc.declare_dram_parameter("dmask", [128, KB * QB], CD, isOutput=False)
    else:
        maskT = nc.declare_dram_parameter("maskT", [S, S], F32, isOutput=False)
    out_c = nc.declare_dram_parameter("out", [TC, H], F32, isOutput=True)

    with tile.TileContext(nc) as tc:
        with tc.tile_pool(name="persist", bufs=1) as per, \
             tc.tile_pool(name="dram", bufs=1, space="DRAM") as dram:
            # persistent SBUF tensors
            qt = per.tile([128, QHC * T], CD)      # rope'd Q, head-major [d, t]
            kt = per.tile([128, T], CD)            # rope'd K [d, t]
            vt = per.tile([128, T], CD)            # V tiles [t(128), d] at col k*128
            cs = per.tile([128, S], F32)
            sg = per.tile([128, S], F32)
            ones = per.tile([128, 128], CD)
            ones_f32 = per.tile([128, 1], F32)
            ones_fr = per.tile([128, 1], mybir.dt.float32r)
            nc.sync.dma_start(cs[:], cosT[:])
            nc.sync.dma_start(sg[:], sgnT[:])
            nc.gpsimd.memset(ones[:], 1.0)
            nc.gpsimd.memset(ones_f32[:], 1.0)
            nc.scalar.activation(ones_fr[:], ones_f32[:], AF.Copy)
            if causal:
                dm = per.tile([128, KB * QB], CD)
                nc.sync.dma_start(dm[:], dmask[:])

            # per-head A2A bounce buffers
            a2a_in = [dram.tile([n_cores, 128, TC], CD, name=f"a2a_in{i}")
                      for i in range(QHC)]
            a2a_out = [dram.tile([n_cores, 128, TC], CD, name=f"a2a_out{i}")
                       for i in range(QHC)]

            # ---------------- Phase 1: QKV projections + RoPE ----------------
            with tc.tile_pool(name="wq", bufs=1) as wqp, \
                 tc.tile_pool(name="hst", bufs=3) as hstp, \
                 tc.tile_pool(name="qkps", bufs=2, space="PSUM") as qkps, \
                 tc.tile_pool(name="vps", bufs=2, space="PSUM") as vps, \
                 tc.tile_pool(name="epi", bufs=3) as epi:
                wq_sb = wqp.tile([128, HT * D], CD)
                wk_sb = wqp.tile([128, HT * HD], CD)
                wv_sb = wqp.tile([128, HT * HD], CD)
                nc.sync.dma_start(
                    wq_sb[:].rearrange("p (ht d) -> p ht d", ht=HT),
                    wqT.rearrange("(ht p) d -> p ht d", p=128))
                nc.sync.dma_start(
                    wk_sb[:].rearrange("p (ht d) -> p ht d", ht=HT),
                    wkT.rearrange("(ht p) d -> p ht d", p=128))
                nc.sync.dma_start(
                    wv_sb[:].rearrange("p (ht d) -> p ht d", ht=HT),
                    wvT.rearrange("(ht p) d -> p ht d", p=128))

                def rope(pA, pB, dst_ap, sc):
                    # dst = ab*cos + rotate_half(ab)*sin with ab = pA + pB.
                    # The half-rotation crosses partitions, which compute
                    # engines cannot do SBUF->SBUF, so shift via on-chip DMA.
                    ab = epi.tile([128, TI], F32, tag="ab", name="ab")
                    nc.scalar.activation(ab[:], pA[:], AF.Copy)
                    nc.vector.tensor_add(ab[:], ab[:], pB[:])
                    sh = epi.tile([128, TI], F32, tag="sh", name="sh")
                    nc.sync.dma_start(sh[0:64, :], ab[64:128, :])
                    nc.sync.dma_start(sh[64:128, :], ab[0:64, :])
                    x1 = epi.tile([128, TI], F32, tag="x1", name="x1")
                    nc.vector.tensor_mul(x1[:], ab[:], cs[:, sc:sc + TI])
                    nc.vector.tensor_mul(sh[:], sh[:], sg[:, sc:sc + TI])
                    nc.vector.tensor_add(dst_ap, x1[:], sh[:])

                for it in range(NIT):
                    t0 = it * TI
                    sc = t0 % S  # column into cos/sgn tables
                    hs_ts = []
                    nh2 = HT // 2
                    for half in range(2):
                        hsp = hstp.tile([128, nh2 * TI], CD, tag="hsp",
                                        name=f"hsp_{it}_{half}")
                        nc.sync.dma_start(
                            hsp[:].rearrange("p (ht t) -> p ht t", ht=nh2),
                            hsT[half * nh2 * 128:(half + 1) * nh2 * 128,
                                t0:t0 + TI].rearrange("(ht p) t -> p ht t",
                                                      p=128))
                        for j in range(nh2):
                            hs_ts.append(hsp[:, j * TI:(j + 1) * TI])
                    # q heads + k: accumulate over ht alternating two PSUM
                    # banks (avoids same-bank drain serialization), then
                    # combine A+B in the epilogue.
                    for g in range(QHC + 1):  # 4 q heads then k
                        pA = qkps.tile([128, TI], F32, tag="pA",
                                       name=f"pA_{it}_{g}")
                        pB = qkps.tile([128, TI], F32, tag="pB",
                                       name=f"pB_{it}_{g}")
                        for ht in range(HT):
                            if g < QHC:
                                w_ap = wq_sb[:, ht * D + g * 128: ht * D + (g + 1) * 128]
                            else:
                                w_ap = wk_sb[:, ht * HD:(ht + 1) * HD]
                            dst = pA if ht % 2 == 0 else pB
                            nc.tensor.matmul(dst[:], w_ap, hs_ts[ht],
                                             start=(ht < 2), stop=(ht >= HT - 2))
                        if g < QHC:
                            rope(pA, pB, qt[:, g * T + t0: g * T + t0 + TI], sc)
                        else:
                            rope(pA, pB, kt[:, t0:t0 + TI], sc)
                    # v: [t,128] x wv groups; N=128 matmuls are drain-free
                    for tsub in range(TI // 128):
                        vp = vps.tile([128, 128], F32, tag="vp",
                                      name=f"vp_{it}_{tsub}")
                        for ht in range(HT):
                            nc.tensor.matmul(
                                vp[:], hs_ts[ht][:, tsub * 128:(tsub + 1) * 128],
                                wv_sb[:, ht * HD:(ht + 1) * HD],
                                start=(ht == 0), stop=(ht == HT - 1))
                        nc.scalar.activation(
                            vt[:, t0 + tsub * 128: t0 + (tsub + 1) * 128],
                            vp[:], AF.Copy)

            tc.no_sync_barrier()
            # ---------------- Phase 2: attention ----------------
            # ST pairs: two k-tiles share one [128, 2*QB] PSUM tile (two
            # banks), one exp per pair; causal masking multiplies the exp
            # output by a 0/1 pattern (cheap bf16 4x DVE); denominator rows
            # accumulate on PE via M=1 matmuls against the bf16 exp tiles.
            with tc.tile_pool(name="stps", bufs=2, space="PSUM") as stps, \
                 tc.tile_pool(name="otps", bufs=2, space="PSUM") as otps, \
                 tc.tile_pool(name="dbps", bufs=2, space="PSUM") as dbps, \
                 tc.tile_pool(name="att", bufs=6) as att, \
                 tc.tile_pool(name="acc", bufs=2) as accp, \
                 tc.tile_pool(name="attm", bufs=3) as attm:
                for hl in range(QHC):
                    for b in range(B):
                        for qb in range(NQB):
                            q0 = b * S + qb * QB          # global q col
                            n_k = (qb + 1) * KB if causal else NKT
                            otp = otps.tile([128, QB], F32, tag="ot",
                                            name=f"ot_{hl}_{b}_{qb}")
                            acc = accp.tile([128, QB], mybir.dt.float32r,
                                            tag="acc",
                                            name=f"acc_{hl}_{b}_{qb}")
                            for kp in range(n_k // 2):
                                stp = stps.tile([128, 2 * QB], F32, tag="st",
                                                name=f"st_{hl}_{b}_{qb}_{kp}")
                                for half in range(2):
                                    kti = 2 * kp + half
                                    kg = b * NKT + kti
                                    nc.tensor.matmul(
                                        stp[:, half * QB:(half + 1) * QB],
                                        kt[:, kg * 128:(kg + 1) * 128],
                                        qt[:, hl * T + q0: hl * T + q0 + QB],
                                        start=True, stop=True)
                                pt = att.tile([128, 2 * QB], CD, tag="pt",
                                              name=f"pt_{hl}_{b}_{qb}_{kp}")
                                d0 = 2 * kp - qb * KB  # diag pattern index
                                if causal and 2 * kp + 1 >= qb * KB:
                                    pr = att.tile([128, 2 * QB], CD, tag="pr",
                                                  name="pr")
                                    nc.scalar.activation(pr[:], stp[:], AF.Exp,
                                                         scale=ISQ)
                                    nc.vector.tensor_mul(
                                        pt[:], pr[:],
                                        dm[:, d0 * QB:(d0 + 2) * QB])
                                elif not causal:
                                    mt = attm.tile([128, 2 * QB], F32, tag="mt",
                                                   name="mt")
                                    for half in range(2):
                                        kti = 2 * kp + half
                                        nc.sync.dma_start(
                                            mt[:, half * QB:(half + 1) * QB],
                                            maskT[kti * 128:(kti + 1) * 128,
                                                  qb * QB:(qb + 1) * QB])
                                    tmp = att.tile([128, 2 * QB], F32, tag="tmp",
                                                   name="tmp")
                                    nc.vector.tensor_add(tmp[:], stp[:], mt[:])
                                    nc.scalar.activation(pt[:], tmp[:], AF.Exp,
                                                         scale=ISQ)
                                else:
                                    nc.scalar.activation(pt[:], stp[:], AF.Exp,
                                                         scale=ISQ)
                                for half in range(2):
                                    kti = 2 * kp + half
                                    kg = b * NKT + kti
                                    nc.tensor.matmul(
                                        otp[:], vt[:, kg * 128:(kg + 1) * 128],
                                        pt[:, half * QB:(half + 1) * QB],
                                        start=(kti == 0), stop=(kti == n_k - 1))
                                if kp == 0:
                                    nc.vector.tensor_add(
                                        acc[:], pt[:, 0:QB], pt[:, QB:2 * QB])
                                else:
                                    nc.vector.tensor_add(
                                        acc[:], acc[:], pt[:, 0:QB])
                                    nc.vector.tensor_add(
                                        acc[:], acc[:], pt[:, QB:2 * QB])
                            db = dbps.tile([128, QB], F32, tag="db",
                                           name=f"db_{hl}_{b}_{qb}")
                            nc.tensor.matmul(db[0:1, :], ones_fr[:], acc[:],
                                             start=True, stop=True)
                            dsb = att.tile([1, QB], CD, tag="dsb", name="dsb")
                            nc.vector.tensor_copy(dsb[:], db[0:1, :])
                            nc.tensor.matmul(db[:], ones[0:1, :], dsb[:],
                                             start=True, stop=True)
                            rec = att.tile([128, QB], F32, tag="rec", name="rec")
                            nc.vector.reciprocal(rec[:], db[:])
                            ot_sb = att.tile([128, QB], CD, tag="otsb",
                                             name="otsb")
                            nc.vector.tensor_mul(ot_sb[:], otp[:], rec[:])
                            # scatter into this head's A2A input buffer
                            nj = max(1, QB // TC)
                            j0 = q0 // TC
                            c0 = q0 % TC
                            if nj == 1:
                                nc.sync.dma_start(
                                    a2a_in[hl][j0, :, c0:c0 + QB], ot_sb[:])
                            else:
                                nc.sync.dma_start(
                                    a2a_in[hl][j0:j0 + nj, :, :],
                                    ot_sb.rearrange("p (j c) -> j p c", j=nj))
                    nc.gpsimd.collective_compute(
                        "AllToAll", ALU.bypass,
                        replica_groups=[list(range(n_cores))],
                        ins=[a2a_in[hl][:]],
                        outs=[a2a_out[hl][:]])

            tc.no_sync_barrier()
            # ---------------- Phase 3: output projection ----------------
            with tc.tile_pool(name="otl", bufs=1) as otlp, \
                 tc.tile_pool(name="wot", bufs=8) as wotp, \
                 tc.tile_pool(name="ops", bufs=1, space="PSUM") as ops, \
                 tc.tile_pool(name="oout", bufs=4) as ooutp:
                ot_loc = otlp.tile([128, NH * TC], CD)
                d_order = [i * QHC + hl for hl in range(QHC) for i in range(n_cores)]
                for g in d_order:
                    nc.sync.dma_start(
                        ot_loc[:, g * TC:(g + 1) * TC],
                        a2a_out[g % QHC][g // QHC, :, :])
                for mp in range(NMB // 2):
                    pos = [ops.tile([128, 512], F32, tag=f"po{sub}{tt}",
                                    name=f"po_{mp}_{sub}_{tt}")
                           for sub in range(2) for tt in range(NT)]
                    for gi, g in enumerate(d_order):
                        wot = wotp.tile([128, 1024], CD, tag="wot",
                                        name=f"wot_{mp}_{g}")
                        nc.sync.dma_start(
                            wot[:], woT[g * 128:(g + 1) * 128,
                                        mp * 1024:(mp + 1) * 1024])
                        for sub in range(2):
                            for tt in range(NT):
                                nc.tensor.matmul(
                                    pos[sub * NT + tt],
                                    ot_loc[:, g * TC + tt * 128: g * TC + (tt + 1) * 128],
                                    wot[:, sub * 512:(sub + 1) * 512],
                                    start=(gi == 0), stop=(gi == NH - 1))
                    for sub in range(2):
                        for tt in range(NT):
                            ob = ooutp.tile([128, 512], F32, tag="ob", name="ob")
                            nc.scalar.activation(ob[:], pos[sub * NT + tt],
                                                 AF.Copy)
                            nc.sync.dma_start(
                                out_c[tt * 128:(tt + 1) * 128,
                                      (2 * mp + sub) * 512:(2 * mp + sub + 1) * 512],
                                ob[:])

    nc.compile()
    return nc


def _np16(x):
    return np.asarray(x, dtype=ml_dtypes.bfloat16)


def prep_inputs(hidden_states, attention_mask, cos, sin, wq, wk, wv, wo,
                compute_dtype="bf16", n_cores=N_CORES):
    """Host-side sharding + transposes. Returns (in_maps, causal, dims)."""
    B, S, H = hidden_states.shape
    T = B * S
    D = NH * HD // n_cores
    KD = NKV * HD // n_cores
    cd = (lambda x: _np16(x)) if compute_dtype == "bf16" else \
         (lambda x: np.ascontiguousarray(x, dtype=np.float32))

    hs2 = np.asarray(hidden_states, np.float32).reshape(T, H)
    hsT = cd(hs2.T)
    woT = cd(np.asarray(wo, np.float32).T)
    cosT = np.ascontiguousarray(np.asarray(cos, np.float32)[0].T)
    sinT = np.ascontiguousarray(np.asarray(sin, np.float32)[0].T)
    sgnT = np.concatenate([-sinT[0:HD // 2], sinT[HD // 2:]], axis=0)
    sgnT = np.ascontiguousarray(sgnT)

    m = np.asarray(attention_mask, np.float32)[0, 0]
    expected = np.where(np.tril(np.ones((S, S), bool)), 0.0, NEG).astype(np.float32)
    causal = bool(np.array_equal(m, expected))

    TC = T // n_cores
    QB = min(512, TC)
    KB = QB // 128
    in_maps = []
    for c in range(n_cores):
        im = {
            "hsT": hsT,
            "wqT": cd(np.asarray(wq, np.float32)[c * D:(c + 1) * D].T),
            "wkT": cd(np.asarray(wk, np.float32)[c * KD:(c + 1) * KD].T),
            "wvT": cd(np.asarray(wv, np.float32)[c * KD:(c + 1) * KD].T),
            "woT": woT,
            "cosT": cosT,
            "sgnT": sgnT,
        }
        if causal:
            pk = np.arange(128)[:, None]
            pq = np.arange(QB)[None, :]
            dmask = np.concatenate(
                [np.where(pk + j * 128 <= pq, 1.0, 0.0) for j in range(KB)],
                axis=1).astype(np.float32)
            im["dmask"] = cd(dmask)
        else:
            im["maskT"] = np.ascontiguousarray(m.T)
        in_maps.append(im)
    return in_maps, causal, (T, S, H)


_BUILD_CACHE = {}


def kernel(hidden_states, attention_mask, cos, sin, wq, wk, wv, wo,
           compute_dtype="bf16", trace=False):
    B, S, H = hidden_states.shape
    T = B * S
    in_maps, causal, dims = prep_inputs(
        hidden_states, attention_mask, cos, sin, wq, wk, wv, wo,
        compute_dtype=compute_dtype)
    key = (T, S, H, compute_dtype, causal)
    if key not in _BUILD_CACHE:
        _BUILD_CACHE[key] = build(T, S, H, compute_dtype=compute_dtype,
                                  causal=causal)
    nc = _BUILD_CACHE[key]
    res = run_bass_kernel_spmd(nc, in_maps, core_ids=list(range(N_CORES)),
                               trace=trace)
    TC = T // N_CORES
    out = np.empty((T, H), np.float32)
    for c in range(N_CORES):
        out[c * TC:(c + 1) * TC] = res.results[c]["out"]
    if trace:
        kernel.last_exec_time_ns = res.exec_time_ns
        kernel.last_results = res
    return out.reshape(B, S, H)

